# revision 5
# baseline (speedup 1.0000x reference)
"""Trainium2 Bass kernel for nn_Combination_ANN_17051020165212.

Strategy:
- Data-parallel over the 16 systems: 2 systems per NeuronCore (8 cores).
- Normalization (Sigma^-1/2 @ (x - mu)) is folded into the first MLP layer
  on the host: W1' = Sigma^T @ W1, b1' = b1 - mu @ W1'.
- The per-(system, shuffle-rep, feature-group) time gather runs on device as
  indirect DMA: each instruction gathers 128 rows' 16B granules (4 features
  of one group) from a small per-(system, group) DRAM table, using offsets
  read one-per-partition from SBUF.
- Gathered [128 rows, 16 feat] tiles are transposed on the PE (identity
  matmul) into [16, 128] feature-major tiles, then the 3-layer MLP runs on
  the PE with Lrelu/Sigmoid on the scalar engine.
- Index streams (identity prefix for the unshuffled block + the permutation
  values) are laid out host-side as [128, n_chunks] so each partition's
  offset for chunk i sits at column i (pure layout transform; the device
  still reads every index byte from HBM).

Output rows per system: 400 unshuffled + 250*400 shuffled = 100400, padded
to 100480 = 785*128 on device; the host trims the padding.
"""

import numpy as np

import bass_rust
import concourse.bass as bass
from concourse.bacc import Bacc
import concourse.mybir as mybir
import concourse.tile as tile
from concourse.bass_utils import run_bass_kernel_spmd
from concourse.masks import make_identity

S, T, F, SF, G = 16, 400, 16, 250, 4
N_CORES = 8
SYS_PER_CORE = S // N_CORES
ROWS = T + SF * T          # 100400 valid rows per system
CHUNKS = (ROWS + 127) // 128  # 785
ROWS_PAD = CHUNKS * 128    # 100480
TAB_PAD = 1024             # slack rows for walrus indirect-AP bounds check

_MAX_WAITS = 1


def _split_excess_waits(nc):
    """This container's walrus rejects >1 sync-wait per instruction; move
    excess waits onto same-engine NOPs inserted right before the owner."""
    for f in nc.m.functions:
        for bb in f.blocks:
            new_insts = []
            for inst in bb.instructions:
                si = inst.sync_info
                waits = list(si.on_wait) if si is not None and si.on_wait else []
                if len(waits) > _MAX_WAITS:
                    excess, keep = waits[:-_MAX_WAITS], waits[-_MAX_WAITS:]
                    si.on_wait = keep
                    for i in range(0, len(excess), _MAX_WAITS):
                        nop = mybir.InstNoOp(
                            name=f"I-waitsplit-{nc.next_id()}", ins=[], outs=[]
                        )
                        nop.engine = inst.engine
                        nop.sync_info = bass_rust.SyncInfo(
                            on_wait=excess[i : i + _MAX_WAITS], on_update=[]
                        )
                        new_insts.append(nop)
                new_insts.append(inst)
            bb.instructions[:] = new_insts


def _build_nc():
    nc = Bacc()
    f32, i32 = mybir.dt.float32, mybir.dt.int32

    pidx = nc.dram_tensor("pidx", [SYS_PER_CORE, G, 128, CHUNKS], i32, kind="ExternalInput")
    tabs = [
        [
            nc.dram_tensor(f"tab{s}{g}", [T + TAB_PAD, 4], f32, kind="ExternalInput")
            for g in range(G)
        ]
        for s in range(SYS_PER_CORE)
    ]
    w1 = nc.dram_tensor("w1p", [F, 32], f32, kind="ExternalInput")
    b1 = nc.dram_tensor("b1p", [32, 1], f32, kind="ExternalInput")
    w2 = nc.dram_tensor("w2", [32, 16], f32, kind="ExternalInput")
    b2 = nc.dram_tensor("b2", [16, 1], f32, kind="ExternalInput")
    w3 = nc.dram_tensor("w3", [16, 1], f32, kind="ExternalInput")
    b3 = nc.dram_tensor("b3", [1, 1], f32, kind="ExternalInput")
    out = nc.dram_tensor("out", [SYS_PER_CORE, CHUNKS, 128], f32, kind="ExternalOutput")

    with tile.TileContext(nc) as tc:
        with (
            tc.tile_pool(name="const", bufs=1) as cp,
            tc.tile_pool(name="gat", bufs=4) as gp,
            tc.tile_pool(name="act", bufs=4) as ap,
            tc.tile_pool(name="ps", bufs=2, space="PSUM") as pp,
        ):
            ident = cp.tile([128, 128], f32, name="ident")
            make_identity(nc, ident[:])
            w1t = cp.tile([F, 32], f32, name="w1t")
            nc.sync.dma_start(out=w1t[:], in_=w1[:])
            b1t = cp.tile([32, 1], f32, name="b1t")
            nc.sync.dma_start(out=b1t[:], in_=b1[:])
            w2t = cp.tile([32, 16], f32, name="w2t")
            nc.sync.dma_start(out=w2t[:], in_=w2[:])
            b2t = cp.tile([16, 1], f32, name="b2t")
            nc.sync.dma_start(out=b2t[:], in_=b2[:])
            w3t = cp.tile([16, 1], f32, name="w3t")
            nc.sync.dma_start(out=w3t[:], in_=w3[:])
            b3t = cp.tile([1, 1], f32, name="b3t")
            nc.sync.dma_start(out=b3t[:], in_=b3[:])

            its = []
            for s in range(SYS_PER_CORE):
                row = []
                for g in range(G):
                    it = cp.tile([128, CHUNKS], i32, name=f"it{s}{g}")
                    nc.sync.dma_start(out=it[:], in_=pidx[s, g])
                    row.append(it)
                its.append(row)

            for s in range(SYS_PER_CORE):
                with tc.For_i(0, CHUNKS) as i:
                    itcol = gp.tile([128, G], i32, name="itcol")
                    for g in range(G):
                        nc.vector.tensor_copy(
                            out=itcol[:, g : g + 1],
                            in_=its[s][g][:, bass.ds(i, 1)],
                        )
                    gt = gp.tile([128, F], f32, name="gt")
                    for g in range(G):
                        nc.gpsimd.indirect_dma_start(
                            out=gt[:, 4 * g : 4 * g + 4],
                            out_offset=None,
                            in_=tabs[s][g][:],
                            in_offset=bass.IndirectOffsetOnAxis(
                                ap=itcol[:, g : g + 1], axis=0
                            ),
                        )
                    xp = pp.tile([F, 128], f32, name="xp")
                    nc.tensor.transpose(out=xp[:], in_=gt[:], identity=ident[:])
                    xs = ap.tile([F, 128], f32, name="xs")
                    nc.scalar.copy(out=xs[:], in_=xp[:])

                    h1p = pp.tile([32, 128], f32, name="h1p")
                    nc.tensor.matmul(out=h1p[:], lhsT=w1t[:], rhs=xs[:], start=True, stop=True)
                    h1 = ap.tile([32, 128], f32, name="h1")
                    nc.scalar.activation(
                        out=h1[:], in_=h1p[:],
                        func=mybir.ActivationFunctionType.Lrelu,
                        bias=b1t[:], alpha=0.01,
                    )

                    h2p = pp.tile([16, 128], f32, name="h2p")
                    nc.tensor.matmul(out=h2p[:], lhsT=w2t[:], rhs=h1[:], start=True, stop=True)
                    h2 = ap.tile([16, 128], f32, name="h2")
                    nc.scalar.activation(
                        out=h2[:], in_=h2p[:],
                        func=mybir.ActivationFunctionType.Lrelu,
                        bias=b2t[:], alpha=0.01,
                    )

                    op = pp.tile([1, 128], f32, name="op")
                    nc.tensor.matmul(out=op[:], lhsT=w3t[:], rhs=h2[:], start=True, stop=True)
                    ot = ap.tile([1, 128], f32, name="ot")
                    nc.scalar.activation(
                        out=ot[:], in_=op[:],
                        func=mybir.ActivationFunctionType.Sigmoid,
                        bias=b3t[:],
                    )
                    nc.sync.dma_start(out=out[s, bass.ds(i, 1), :], in_=ot[:])
    nc.finalize()
    try:
        nc.thaw()
    except Exception:
        pass
    _split_excess_waits(nc)
    try:
        nc.freeze()
    except Exception:
        pass
    return nc


_NC_CACHE = None


def _get_nc():
    global _NC_CACHE
    if _NC_CACHE is None:
        _NC_CACHE = _build_nc()
    return _NC_CACHE


def kernel(
    observations, mu, Sigma_minus_half, perm_idx, W1, b1, W2, b2, W3, b3
):
    observations = np.asarray(observations, dtype=np.float32)
    mu = np.asarray(mu, dtype=np.float32)
    Sigma_minus_half = np.asarray(Sigma_minus_half, dtype=np.float32)
    perm_idx = np.asarray(perm_idx, dtype=np.int32)
    W1 = np.asarray(W1, dtype=np.float32)
    b1 = np.asarray(b1, dtype=np.float32)
    W2 = np.asarray(W2, dtype=np.float32)
    b2 = np.asarray(b2, dtype=np.float32)
    W3 = np.asarray(W3, dtype=np.float32)
    b3 = np.asarray(b3, dtype=np.float32)

    # Fold whitening into layer 1.
    W1p = (Sigma_minus_half.T @ W1).astype(np.float32)  # [F, 32]
    b1p = (b1 - mu[:, 0] @ W1p).astype(np.float32)

    # Index streams: identity prefix (unshuffled block), then the per-
    # (system, group) permutation values, zero padding to a whole chunk
    # count; laid out [128, CHUNKS] column-per-chunk.
    iota = np.arange(T, dtype=np.int32)
    pad = np.zeros(ROWS_PAD - ROWS, dtype=np.int32)

    in_maps = []
    for c in range(N_CORES):
        m = {}
        pidx = np.empty((SYS_PER_CORE, G, 128, CHUNKS), np.int32)
        for s2 in range(SYS_PER_CORE):
            s = SYS_PER_CORE * c + s2
            for g in range(G):
                stream = np.concatenate([iota, perm_idx[:, g, s, :].ravel(), pad])
                pidx[s2, g] = stream.reshape(CHUNKS, 128).T
                tab = np.zeros((T + TAB_PAD, 4), np.float32)
                tab[:T] = observations[s, :, 4 * g : 4 * g + 4]
                m[f"tab{s2}{g}"] = tab
        m["pidx"] = pidx
        m["w1p"] = W1p
        m["b1p"] = b1p[:, None].copy()
        m["w2"] = W2
        m["b2"] = b2[:, None].copy()
        m["w3"] = W3
        m["b3"] = b3[:, None].copy()
        in_maps.append(m)

    nc = _get_nc()
    res = run_bass_kernel_spmd(nc, in_maps, list(range(N_CORES)))

    out = np.empty((S, ROWS, 1), np.float32)
    for c in range(N_CORES):
        o = res.results[c]["out"].reshape(SYS_PER_CORE, ROWS_PAD)
        for s2 in range(SYS_PER_CORE):
            out[SYS_PER_CORE * c + s2, :, 0] = o[s2, :ROWS]
    return out
